# revision 1
# baseline (speedup 1.0000x reference)
"""GRNN (nn_GRNN_71502615544225) Trainium2 kernel, 8-way sharded over train set.

Math: out[b] = sum_n w[b,n]*y[n] / sum_n w[b,n],  w = exp(-||x_b-t_n||^2/(2s^2)).
The per-row factor exp(x_b^2/(2s^2)) cancels in the ratio, so each core computes
exponent[b,n] = x_b.(t_n/s^2) - t_n^2/(2s^2)  via matmul with an augmented
contraction, exp on the scalar engine, then a second matmul against
[train_outputs | 1] to get partial weighted sums + weight sums. Host adds the
8 partial results and divides.

fp32 x.t precision at bf16 matmul speed via hi/lo split:
  x.t' = (xh+xl).th' (one K=128 stacked matmul) + xh.tl' (K=66 matmul that also
  carries the -t^2/(2s^2) bias rows). Dropped xl.tl' term ~ 3e-4 absolute.
"""
import numpy as np
import ml_dtypes

import concourse.bacc as bacc
import concourse.mybir as mybir
import concourse.tile as tile
from concourse.bass_utils import run_bass_kernel_spmd

F32 = mybir.dt.float32
BF = mybir.dt.bfloat16

B, D, O, N = 2048, 64, 16, 100000
NCORES = 8
NS = N // NCORES            # 12500 train rows per core
CH = (NS + 127) // 128      # 98 chunks of 128 rows
NSP = CH * 128              # 12544 padded rows
BSL = B // 512              # 4 b-slices of 512
M_SLICES = CH * BSL         # 392 (chunk, b-slice) matmul slices
# exp windows alternate 4-slice (2048) / 3-slice (1536): 7 PSUM banks for
# the exponent staging + 1 for the output accumulator; fewer, larger
# activation instructions amortize the ~352-cycle ACT issue overhead.
GRP = 7                     # slices per window pair
NWIN = 2 * (M_SLICES // GRP)        # 112 windows (56 pairs)


def win_of(m):
    g, r = divmod(m, GRP)
    return (2 * g, r) if r < 4 else (2 * g + 1, r - 4)
# t-tile piece sizes in chunks: small first piece so compute starts early
PIECES = [2] + [12] * 8          # sums to 98
POFF = [0]
for _p in PIECES:
    POFF.append(POFF[-1] + _p)

_prog_cache = {}


def build_program(repeat=1):
    # repeat>1 replays the compute loop on the same SBUF data (benchmarking
    # aid: num/den both scale by `repeat`, so the final ratio is unchanged).
    if repeat in _prog_cache:
        return _prog_cache[repeat]
    nc = bacc.Bacc("TRN2", target_bir_lowering=False, debug=False,
                   num_devices=NCORES)
    xc_d = nc.dram_tensor("xc", [128, 4096], BF, kind="ExternalInput").ap()
    ta_d = nc.dram_tensor("ta", [128, NSP], BF, kind="ExternalInput").ap()
    tb_d = nc.dram_tensor("tb", [66, NSP], BF, kind="ExternalInput").ap()
    to_d = nc.dram_tensor("to", [128, CH * 17], BF, kind="ExternalInput").ap()
    out_d = nc.dram_tensor("out", [17, B], F32, kind="ExternalOutput").ap()

    with tile.TileContext(nc) as tc:
        with (
            tc.tile_pool(name="const", bufs=1) as cpool,
            tc.tile_pool(name="tap", bufs=1) as tapool,
            tc.tile_pool(name="tbp", bufs=1) as tbpool,
            tc.tile_pool(name="wring", bufs=6) as wpool,
            tc.tile_pool(name="s4pool", bufs=1, space="PSUM") as s4pool,
            tc.tile_pool(name="s3pool", bufs=1, space="PSUM") as s3pool,
            tc.tile_pool(name="apool", bufs=1, space="PSUM") as apool,
        ):
            # per-b-slice x tiles so the first matmul waits on ~0.2MB only.
            # DMA issue order = consumption order (critical path first).
            xa_t = [cpool.tile([128, 512], BF, tag=f"xa{j}", name=f"xa{j}")
                    for j in range(BSL)]
            xb_t = [cpool.tile([66, 512], BF, tag=f"xb{j}", name=f"xb{j}")
                    for j in range(BSL)]
            ta_t = [tapool.tile([128, np_ * 128], BF, tag=f"ta{k}",
                                name=f"ta{k}")
                    for k, np_ in enumerate(PIECES)]
            tb_t = [tbpool.tile([66, np_ * 128], BF, tag=f"tb{k}",
                                name=f"tb{k}")
                    for k, np_ in enumerate(PIECES)]
            to_t = cpool.tile([128, CH * 17], BF)

            def _load_piece(k):
                w0, w1 = POFF[k] * 128, POFF[k + 1] * 128
                nc.sync.dma_start(ta_t[k][:], ta_d[:, w0:w1])
                nc.sync.dma_start(tb_t[k][:], tb_d[:, w0:w1])

            nc.sync.dma_start(xa_t[0][:], xc_d[:, 0:512])
            nc.sync.dma_start(xb_t[0][:], xc_d[0:66, 2048:2560])
            _load_piece(0)
            for j in range(1, BSL):
                nc.sync.dma_start(
                    xa_t[j][:], xc_d[:, j * 512:(j + 1) * 512])
                nc.sync.dma_start(
                    xb_t[j][:],
                    xc_d[0:66, 2048 + j * 512:2048 + (j + 1) * 512])
            nc.sync.dma_start(to_t[:], to_d)
            for k in range(1, len(PIECES)):
                _load_piece(k)

            def t_slice(tiles, i):
                for k in range(len(PIECES)):
                    if i < POFF[k + 1]:
                        kk = i - POFF[k]
                        return tiles[k][:, kk * 128:(kk + 1) * 128]
                raise AssertionError

            acc = apool.tile([128, 512], F32)
            stile = None
            next_mm2 = 0

            total_ch = CH * repeat
            ring = [None] * (NWIN * repeat)
            for m in range(M_SLICES * repeat):
                i, j = divmod(m, BSL)
                i = i % CH
                w, pos = win_of(m)
                nsl = 4 if w % 2 == 0 else 3
                if pos == 0:
                    if nsl == 4:
                        stile = s4pool.tile([128, 4 * 512], F32, tag="s4",
                                            name="s4")
                    else:
                        stile = s3pool.tile([128, 3 * 512], F32, tag="s3",
                                            name="s3")
                ssl = stile[:, pos * 512:(pos + 1) * 512]
                # exponent = (xh+xl).th'  +  (xh.tl' + 1*tsq_h + 1*tsq_l)
                nc.tensor.matmul(
                    ssl, t_slice(ta_t, i), xa_t[j][:],
                    start=True, stop=False)
                nc.tensor.matmul(
                    ssl, t_slice(tb_t, i)[0:66], xb_t[j][:],
                    start=False, stop=True)

                last = m == M_SLICES * repeat - 1
                if pos == nsl - 1:
                    width = nsl * 512
                    wt = wpool.tile([128, 4 * 512], BF, tag="wt")
                    nc.scalar.activation(
                        wt[:, :width], stile[:, :width],
                        mybir.ActivationFunctionType.Exp)
                    ring[w] = wt
                    # 2nd matmul for chunks whose exp windows completed two
                    # windows ago: the lag keeps the in-order PE queue from
                    # stalling on the just-issued Exp (wring bufs give slack).
                    while (next_mm2 < total_ch
                           and win_of(4 * next_mm2 + 3)[0] <= (w - 2
                                if not last else w)):
                        ic = next_mm2
                        icm = ic % CH
                        for j2 in range(BSL):
                            m2 = 4 * ic + j2
                            w2, pos2 = win_of(m2)
                            nc.tensor.matmul(
                                acc[32 * j2:32 * j2 + 17, :],
                                to_t[:, 17 * icm:17 * icm + 17],
                                ring[w2][:, pos2 * 512:(pos2 + 1) * 512],
                                start=(ic == 0), stop=(ic == total_ch - 1),
                                tile_position=(0, 32 * j2))
                        next_mm2 += 1

            res = cpool.tile([128, 512], F32)
            for j2 in range(BSL):
                nc.vector.tensor_copy(
                    res[32 * j2:32 * j2 + 17, :], acc[32 * j2:32 * j2 + 17, :])
                nc.sync.dma_start(
                    out_d[:, 512 * j2:512 * (j2 + 1)],
                    res[32 * j2:32 * j2 + 17, :])
    nc.compile()
    _prog_cache[repeat] = nc
    return nc


def _bf(x):
    return np.asarray(x, dtype=ml_dtypes.bfloat16)


def host_prep(x, train_inputs, train_outputs, spread):
    x = np.asarray(x, np.float32)
    t = np.asarray(train_inputs, np.float32)
    y = np.asarray(train_outputs, np.float32)
    s = np.float32(1.0) / (2.0 * np.float32(spread[0]) ** 2)

    tp = (t * (2.0 * s)).astype(np.float32)          # [N, 64]
    th = _bf(tp)
    tl = _bf(tp - th.astype(np.float32))
    tsq = (-s * np.einsum("nd,nd->n", t, t)).astype(np.float32)
    tsqh = _bf(tsq)
    tsql = _bf(tsq - tsqh.astype(np.float32))
    xh = _bf(x)
    xl = _bf(x - xh.astype(np.float32))

    xc = np.zeros((128, 4096), dtype=ml_dtypes.bfloat16)
    xc[0:64, 0:2048] = xh.T
    xc[64:128, 0:2048] = xl.T
    xc[0:64, 2048:4096] = xh.T
    xc[64:66, 2048:4096] = _bf(np.ones((2, 2048), np.float32))

    in_maps = []
    for c in range(NCORES):
        n0 = c * NS
        ta = np.zeros((128, NSP), dtype=ml_dtypes.bfloat16)
        ta[0:64, :NS] = th[n0:n0 + NS].T
        ta[64:128, :NS] = th[n0:n0 + NS].T
        tb = np.zeros((66, NSP), dtype=ml_dtypes.bfloat16)
        tb[0:64, :NS] = tl[n0:n0 + NS].T
        tb[64, :NS] = tsqh[n0:n0 + NS]
        tb[65, :NS] = tsql[n0:n0 + NS]
        tb[64, NS:] = _bf(np.float32(-1e30))  # pad rows -> exp(-1e30) = 0
        to = np.zeros((NSP, 17), dtype=np.float32)
        to[:NS, :16] = y[n0:n0 + NS]
        to[:, 16] = 1.0
        to[NS:, 16] = 0.0
        # sbuf layout [p, 17*o+f] with n = 128*o + p
        to_r = _bf(to.reshape(CH, 128, 17).transpose(1, 0, 2).reshape(128, CH * 17))
        in_maps.append({"xc": xc, "ta": ta, "tb": tb, "to": to_r})
    return in_maps


def run_cores(in_maps, trace=False, repeat=1, **kw):
    nc = build_program(repeat)
    return run_bass_kernel_spmd(nc, in_maps, list(range(NCORES)),
                                trace=trace, **kw)


def kernel(x, train_inputs, train_outputs, spread):
    in_maps = host_prep(x, train_inputs, train_outputs, spread)
    res = run_cores(in_maps)
    total = np.zeros((17, B), dtype=np.float64)
    for c in range(NCORES):
        total += res.results[c]["out"].astype(np.float64)
    out = (total[:16] / total[16]).T.astype(np.float32)
    return out



# revision 2
# speedup vs baseline: 1.2231x; 1.2231x over previous
"""GRNN (nn_GRNN_71502615544225) Trainium2 kernel, 8-way sharded over train set.

Math: out[b] = sum_n w[b,n]*y[n] / sum_n w[b,n],  w = exp(-||x_b-t_n||^2/(2s^2)).
The per-row factor exp(x_b^2/(2s^2)) cancels in the ratio, so each core computes
exponent[b,n] = x_b.(t_n/s^2) - t_n^2/(2s^2)  via ONE K=128 bf16 matmul, exp on
the scalar engine, then a second matmul against [train_outputs | 1] to get
partial weighted sums + weight sums. Host adds the 8 partial results and
divides.

K=128 contraction layout (t-side rows | x-side rows):
  rows   0..63 : th (bf16 of t/s^2)       | xh (bf16 hi of x)
  rows  64..125: th dims 0..61            | xl dims 0..61 (bf16 lo of x)
  row   126    : tsqh (bf16 hi of -t^2/2s^2, from th for consistency) | 1
  row   127    : tsql (bf16 lo)           | 1
So exponent = (xh + xl[0:62]).th + tsq with t rounded to bf16; dropping the
xh.tl correction costs ~1e-2 rel err (vs 2e-2 gate), dropping xl dims 62/63
costs ~3e-3 in exponent std — both validated against the reference inputs.
"""
import numpy as np
import ml_dtypes

import concourse.bacc as bacc
import concourse.mybir as mybir
import concourse.tile as tile
from concourse.bass_utils import run_bass_kernel_spmd

F32 = mybir.dt.float32
BF = mybir.dt.bfloat16

B, D, O, N = 2048, 64, 16, 100000
NCORES = 8
NS = N // NCORES            # 12500 train rows per core
CH = (NS + 127) // 128      # 98 chunks of 128 rows
NSP = CH * 128              # 12544 padded rows
BSL = B // 512              # 4 b-slices of 512
M_SLICES = CH * BSL         # 392 (chunk, b-slice) matmul slices
# exp windows alternate 4-slice (2048) / 3-slice (1536): 7 PSUM banks for
# the exponent staging + 1 for the output accumulator; fewer, larger
# activation instructions amortize the ~352-cycle ACT issue overhead.
GRP = 7                     # slices per window pair
NWIN = 2 * (M_SLICES // GRP)        # 112 windows (56 pairs)


def win_of(m):
    g, r = divmod(m, GRP)
    return (2 * g, r) if r < 4 else (2 * g + 1, r - 4)
# t-tile piece sizes in chunks: small first piece so compute starts early
PIECES = [2] + [12] * 8          # sums to 98
POFF = [0]
for _p in PIECES:
    POFF.append(POFF[-1] + _p)

_prog_cache = {}


def build_program(repeat=1):
    # repeat>1 replays the compute loop on the same SBUF data (benchmarking
    # aid: num/den both scale by `repeat`, so the final ratio is unchanged).
    if repeat in _prog_cache:
        return _prog_cache[repeat]
    nc = bacc.Bacc("TRN2", target_bir_lowering=False, debug=False,
                   num_devices=NCORES)
    xc_d = nc.dram_tensor("xc", [128, 2048], BF, kind="ExternalInput").ap()
    ta_d = nc.dram_tensor("ta", [128, NSP], BF, kind="ExternalInput").ap()
    to_d = nc.dram_tensor("to", [128, CH * 17], BF, kind="ExternalInput").ap()
    out_d = nc.dram_tensor("out", [17, B], F32, kind="ExternalOutput").ap()

    with tile.TileContext(nc) as tc:
        with (
            tc.tile_pool(name="const", bufs=1) as cpool,
            tc.tile_pool(name="tap", bufs=1) as tapool,
            tc.tile_pool(name="wring", bufs=6) as wpool,
            tc.tile_pool(name="s4pool", bufs=1, space="PSUM") as s4pool,
            tc.tile_pool(name="s3pool", bufs=1, space="PSUM") as s3pool,
            tc.tile_pool(name="apool", bufs=1, space="PSUM") as apool,
        ):
            # per-b-slice x tiles so the first matmul waits on ~0.2MB only.
            # DMA issue order = consumption order (critical path first).
            xa_t = [cpool.tile([128, 512], BF, tag=f"xa{j}", name=f"xa{j}")
                    for j in range(BSL)]
            ta_t = [tapool.tile([128, np_ * 128], BF, tag=f"ta{k}",
                                name=f"ta{k}")
                    for k, np_ in enumerate(PIECES)]
            to_t = cpool.tile([128, CH * 17], BF)

            def _load_piece(k):
                w0, w1 = POFF[k] * 128, POFF[k + 1] * 128
                nc.sync.dma_start(ta_t[k][:], ta_d[:, w0:w1])

            nc.sync.dma_start(xa_t[0][:], xc_d[:, 0:512])
            _load_piece(0)
            for j in range(1, BSL):
                nc.sync.dma_start(
                    xa_t[j][:], xc_d[:, j * 512:(j + 1) * 512])
            nc.sync.dma_start(to_t[:], to_d)
            for k in range(1, len(PIECES)):
                _load_piece(k)

            def t_slice(tiles, i):
                for k in range(len(PIECES)):
                    if i < POFF[k + 1]:
                        kk = i - POFF[k]
                        return tiles[k][:, kk * 128:(kk + 1) * 128]
                raise AssertionError

            acc = apool.tile([128, 512], F32)
            stile = None
            next_mm2 = 0

            total_ch = CH * repeat
            ring = [None] * (NWIN * repeat)
            for m in range(M_SLICES * repeat):
                i, j = divmod(m, BSL)
                i = i % CH
                w, pos = win_of(m)
                nsl = 4 if w % 2 == 0 else 3
                if pos == 0:
                    if nsl == 4:
                        stile = s4pool.tile([128, 4 * 512], F32, tag="s4",
                                            name="s4")
                    else:
                        stile = s3pool.tile([128, 3 * 512], F32, tag="s3",
                                            name="s3")
                ssl = stile[:, pos * 512:(pos + 1) * 512]
                # exponent = (xh+xl[0:62]).th + tsq, one K=128 matmul
                nc.tensor.matmul(
                    ssl, t_slice(ta_t, i), xa_t[j][:],
                    start=True, stop=True)

                last = m == M_SLICES * repeat - 1
                if pos == nsl - 1:
                    width = nsl * 512
                    wt = wpool.tile([128, 4 * 512], BF, tag="wt")
                    nc.scalar.activation(
                        wt[:, :width], stile[:, :width],
                        mybir.ActivationFunctionType.Exp)
                    ring[w] = wt
                    # 2nd matmul for chunks whose exp windows completed two
                    # windows ago: the lag keeps the in-order PE queue from
                    # stalling on the just-issued Exp (wring bufs give slack).
                    while (next_mm2 < total_ch
                           and win_of(4 * next_mm2 + 3)[0] <= (w - 2
                                if not last else w)):
                        ic = next_mm2
                        icm = ic % CH
                        for j2 in range(BSL):
                            m2 = 4 * ic + j2
                            w2, pos2 = win_of(m2)
                            nc.tensor.matmul(
                                acc[32 * j2:32 * j2 + 17, :],
                                to_t[:, 17 * icm:17 * icm + 17],
                                ring[w2][:, pos2 * 512:(pos2 + 1) * 512],
                                start=(ic == 0), stop=(ic == total_ch - 1),
                                tile_position=(0, 32 * j2))
                        next_mm2 += 1

            res = cpool.tile([128, 512], F32)
            for j2 in range(BSL):
                nc.vector.tensor_copy(
                    res[32 * j2:32 * j2 + 17, :], acc[32 * j2:32 * j2 + 17, :])
                nc.sync.dma_start(
                    out_d[:, 512 * j2:512 * (j2 + 1)],
                    res[32 * j2:32 * j2 + 17, :])
    nc.compile()
    _prog_cache[repeat] = nc
    return nc


def _bf(x):
    return np.asarray(x, dtype=ml_dtypes.bfloat16)


def host_prep(x, train_inputs, train_outputs, spread):
    x = np.asarray(x, np.float32)
    t = np.asarray(train_inputs, np.float32)
    y = np.asarray(train_outputs, np.float32)
    s = np.float32(1.0) / (2.0 * np.float32(spread[0]) ** 2)

    tp = (t * (2.0 * s)).astype(np.float32)          # [N, 64]
    th = _bf(tp)
    # tsq from th (consistent train points) in f32, split to bf16 hi/lo
    thf = th.astype(np.float32)
    tsq = (-(np.einsum("nd,nd->n", thf, thf)) / (4.0 * s)).astype(np.float32)
    tsqh = _bf(tsq)
    tsql = _bf(tsq - tsqh.astype(np.float32))
    xh = _bf(x)
    xl = _bf(x - xh.astype(np.float32))

    xc = np.zeros((128, 2048), dtype=ml_dtypes.bfloat16)
    xc[0:64, :] = xh.T
    xc[64:126, :] = xl.T[0:62]
    xc[126:128, :] = _bf(np.ones((2, 2048), np.float32))

    in_maps = []
    for c in range(NCORES):
        n0 = c * NS
        ta = np.zeros((128, NSP), dtype=ml_dtypes.bfloat16)
        ta[0:64, :NS] = th[n0:n0 + NS].T
        ta[64:126, :NS] = th[n0:n0 + NS].T[0:62]
        ta[126, :NS] = tsqh[n0:n0 + NS]
        ta[127, :NS] = tsql[n0:n0 + NS]
        ta[126, NS:] = _bf(np.float32(-1e30))  # pad rows -> exp(-1e30) = 0
        to = np.zeros((NSP, 17), dtype=np.float32)
        to[:NS, :16] = y[n0:n0 + NS]
        to[:, 16] = 1.0
        to[NS:, 16] = 0.0
        # sbuf layout [p, 17*o+f] with n = 128*o + p
        to_r = _bf(to.reshape(CH, 128, 17).transpose(1, 0, 2).reshape(128, CH * 17))
        in_maps.append({"xc": xc, "ta": ta, "to": to_r})
    return in_maps


def run_cores(in_maps, trace=False, repeat=1, **kw):
    nc = build_program(repeat)
    return run_bass_kernel_spmd(nc, in_maps, list(range(NCORES)),
                                trace=trace, **kw)


def kernel(x, train_inputs, train_outputs, spread):
    in_maps = host_prep(x, train_inputs, train_outputs, spread)
    res = run_cores(in_maps)
    total = np.zeros((17, B), dtype=np.float64)
    for c in range(NCORES):
        total += res.results[c]["out"].astype(np.float64)
    out = (total[:16] / total[16]).T.astype(np.float32)
    return out


# revision 7
# speedup vs baseline: 1.2273x; 1.0034x over previous
"""GRNN (nn_GRNN_71502615544225) Trainium2 kernel, 8-way sharded over train set.

Math: out[b] = sum_n w[b,n]*y[n] / sum_n w[b,n],  w = exp(-||x_b-t_n||^2/(2s^2)).
The per-row factor exp(x_b^2/(2s^2)) cancels in the ratio, so each core computes
exponent[b,n] = x_b.(t_n/s^2) - t_n^2/(2s^2)  via ONE K=128 bf16 matmul, exp on
the scalar engine, then a second matmul against [train_outputs | 1] to get
partial weighted sums + weight sums. Host adds the 8 partial results and
divides.

K=128 contraction layout (t-side rows | x-side rows):
  rows   0..63 : th (bf16 of t/s^2)       | xh (bf16 hi of x)
  rows  64..125: th dims 0..61            | xl dims 0..61 (bf16 lo of x)
  row   126    : tsqh (bf16 hi of -t^2/2s^2, from th for consistency) | 1
  row   127    : tsql (bf16 lo)           | 1
So exponent = (xh + xl[0:62]).th + tsq with t rounded to bf16; dropping the
xh.tl correction costs ~1e-2 rel err (vs 2e-2 gate), dropping xl dims 62/63
costs ~3e-3 in exponent std — both validated against the reference inputs.
"""
import numpy as np
import ml_dtypes

import concourse.bacc as bacc
import concourse.mybir as mybir
import concourse.tile as tile
from concourse.bass_utils import run_bass_kernel_spmd

F32 = mybir.dt.float32
BF = mybir.dt.bfloat16

B, D, O, N = 2048, 64, 16, 100000
NCORES = 8
NS = N // NCORES            # 12500 train rows per core
CH = (NS + 127) // 128      # 98 chunks of 128 rows
NSP = CH * 128              # 12544 padded rows
BSL = B // 512              # 4 b-slices of 512
M_SLICES = CH * BSL         # 392 (chunk, b-slice) matmul slices
# exp windows alternate 4-slice (2048) / 3-slice (1536): 7 PSUM banks for
# the exponent staging + 1 for the output accumulator; fewer, larger
# activation instructions amortize the ~352-cycle ACT issue overhead.
GRP = 7                     # slices per window pair
NWIN = 2 * (M_SLICES // GRP)        # 112 windows (56 pairs)


def win_of(m):
    # 3-slice window first so the scalar engine starts ~1 matmul earlier
    g, r = divmod(m, GRP)
    return (2 * g, r) if r < 3 else (2 * g + 1, r - 3)
# t-tile piece sizes in chunks: small first pieces so compute starts early
PIECES = [1, 2] + [19] * 5          # sums to 98
POFF = [0]
for _p in PIECES:
    POFF.append(POFF[-1] + _p)

_prog_cache = {}


def build_program(repeat=1):
    # repeat>1 replays the compute loop on the same SBUF data (benchmarking
    # aid: num/den both scale by `repeat`, so the final ratio is unchanged).
    if repeat in _prog_cache:
        return _prog_cache[repeat]
    nc = bacc.Bacc("TRN2", target_bir_lowering=False, debug=False,
                   num_devices=NCORES)
    xc_d = nc.dram_tensor("xc", [128, 2048], BF, kind="ExternalInput").ap()
    ta_d = nc.dram_tensor("ta", [128, NSP], BF, kind="ExternalInput").ap()
    to_d = nc.dram_tensor("to", [128, CH * 17], BF, kind="ExternalInput").ap()
    out_d = nc.dram_tensor("out", [17, B], F32, kind="ExternalOutput").ap()

    with tile.TileContext(nc) as tc:
        with (
            tc.tile_pool(name="const", bufs=1) as cpool,
            tc.tile_pool(name="tap", bufs=1) as tapool,
            tc.tile_pool(name="wring", bufs=6) as wpool,
            tc.tile_pool(name="s4pool", bufs=1, space="PSUM") as s4pool,
            tc.tile_pool(name="s3pool", bufs=1, space="PSUM") as s3pool,
            tc.tile_pool(name="apool", bufs=1, space="PSUM") as apool,
        ):
            # per-b-slice x tiles so the first matmul waits on ~0.2MB only.
            # DMA issue order = consumption order (critical path first).
            xa_t = [cpool.tile([128, 512], BF, tag=f"xa{j}", name=f"xa{j}")
                    for j in range(BSL)]
            ta_t = [tapool.tile([128, np_ * 128], BF, tag=f"ta{k}",
                                name=f"ta{k}")
                    for k, np_ in enumerate(PIECES)]
            to_t = cpool.tile([128, CH * 17], BF)

            def _load_piece(k):
                w0, w1 = POFF[k] * 128, POFF[k + 1] * 128
                nc.sync.dma_start(ta_t[k][:], ta_d[:, w0:w1])

            nc.sync.dma_start(xa_t[0][:], xc_d[:, 0:512])
            _load_piece(0)
            for j in range(1, BSL):
                nc.sync.dma_start(
                    xa_t[j][:], xc_d[:, j * 512:(j + 1) * 512])
            _load_piece(1)
            nc.sync.dma_start(to_t[:], to_d)
            for k in range(2, len(PIECES)):
                _load_piece(k)

            def t_slice(tiles, i):
                for k in range(len(PIECES)):
                    if i < POFF[k + 1]:
                        kk = i - POFF[k]
                        return tiles[k][:, kk * 128:(kk + 1) * 128]
                raise AssertionError

            acc = apool.tile([128, 512], F32)
            # PE p-state warmup: ~3us of dummy matmuls (into acc, which the
            # first real mm2 resets with start=True) while input DMAs land,
            # so the first real matmuls run at full clock.
            gtile = cpool.tile([128, 512], BF, name="warmup")
            nc.gpsimd.memset(gtile[:], 0.0)
            for _ in range(6):
                nc.tensor.matmul(acc[:], gtile[:, 0:128], gtile[:],
                                 start=True, stop=True)
            stile = None
            next_mm2 = 0

            total_ch = CH * repeat
            ring = [None] * (NWIN * repeat)
            for m in range(M_SLICES * repeat):
                i, j = divmod(m, BSL)
                i = i % CH
                w, pos = win_of(m)
                nsl = 3 if w % 2 == 0 else 4
                if pos == 0:
                    if nsl == 4:
                        stile = s4pool.tile([128, 4 * 512], F32, tag="s4",
                                            name="s4")
                    else:
                        stile = s3pool.tile([128, 3 * 512], F32, tag="s3",
                                            name="s3")
                ssl = stile[:, pos * 512:(pos + 1) * 512]
                # exponent = (xh+xl[0:62]).th + tsq, one K=128 matmul
                nc.tensor.matmul(
                    ssl, t_slice(ta_t, i), xa_t[j][:],
                    start=True, stop=True)

                last = m == M_SLICES * repeat - 1
                if pos == nsl - 1:
                    width = nsl * 512
                    wt = wpool.tile([128, 4 * 512], BF, tag="wt")
                    nc.scalar.activation(
                        wt[:, :width], stile[:, :width],
                        mybir.ActivationFunctionType.Exp)
                    ring[w] = wt
                    # 2nd matmul for chunks whose exp windows completed two
                    # windows ago: the lag keeps the in-order PE queue from
                    # stalling on the just-issued Exp (wring bufs give slack).
                    while (next_mm2 < total_ch
                           and win_of(4 * next_mm2 + 3)[0] <= (w - 2
                                if not last else w)):
                        ic = next_mm2
                        icm = ic % CH
                        for j2 in range(BSL):
                            m2 = 4 * ic + j2
                            w2, pos2 = win_of(m2)
                            nc.tensor.matmul(
                                acc[32 * j2:32 * j2 + 17, :],
                                to_t[:, 17 * icm:17 * icm + 17],
                                ring[w2][:, pos2 * 512:(pos2 + 1) * 512],
                                start=(ic == 0), stop=(ic == total_ch - 1),
                                tile_position=(0, 32 * j2))
                        next_mm2 += 1

            # Output: copies split across Vector/Scalar engines, DMAs split
            # across the two HW-DGE queues (SP + Activation) — the 17-row
            # bands are descriptor-bound, so two queues halve the tail.
            res = cpool.tile([128, 512], F32)
            for j2 in range(BSL):
                band_r = res[32 * j2:32 * j2 + 17, :]
                band_a = acc[32 * j2:32 * j2 + 17, :]
                if j2 % 2 == 0:
                    nc.vector.tensor_copy(band_r, band_a)
                    nc.sync.dma_start(
                        out_d[:, 512 * j2:512 * (j2 + 1)], band_r)
                else:
                    nc.scalar.copy(band_r, band_a)
                    nc.scalar.dma_start(
                        out_d[:, 512 * j2:512 * (j2 + 1)], band_r)
    nc.compile()
    _prog_cache[repeat] = nc
    return nc


def _bf(x):
    return np.asarray(x, dtype=ml_dtypes.bfloat16)


def host_prep(x, train_inputs, train_outputs, spread):
    x = np.asarray(x, np.float32)
    t = np.asarray(train_inputs, np.float32)
    y = np.asarray(train_outputs, np.float32)
    s = np.float32(1.0) / (2.0 * np.float32(spread[0]) ** 2)

    tp = (t * (2.0 * s)).astype(np.float32)          # [N, 64]
    th = _bf(tp)
    # tsq from th (consistent train points) in f32, split to bf16 hi/lo
    thf = th.astype(np.float32)
    tsq = (-(np.einsum("nd,nd->n", thf, thf)) / (4.0 * s)).astype(np.float32)
    tsqh = _bf(tsq)
    tsql = _bf(tsq - tsqh.astype(np.float32))
    xh = _bf(x)
    xl = _bf(x - xh.astype(np.float32))

    xc = np.zeros((128, 2048), dtype=ml_dtypes.bfloat16)
    xc[0:64, :] = xh.T
    xc[64:126, :] = xl.T[0:62]
    xc[126:128, :] = _bf(np.ones((2, 2048), np.float32))

    in_maps = []
    for c in range(NCORES):
        n0 = c * NS
        ta = np.zeros((128, NSP), dtype=ml_dtypes.bfloat16)
        ta[0:64, :NS] = th[n0:n0 + NS].T
        ta[64:126, :NS] = th[n0:n0 + NS].T[0:62]
        ta[126, :NS] = tsqh[n0:n0 + NS]
        ta[127, :NS] = tsql[n0:n0 + NS]
        ta[126, NS:] = _bf(np.float32(-1e30))  # pad rows -> exp(-1e30) = 0
        to = np.zeros((NSP, 17), dtype=np.float32)
        to[:NS, :16] = y[n0:n0 + NS]
        to[:, 16] = 1.0
        to[NS:, 16] = 0.0
        # sbuf layout [p, 17*o+f] with n = 128*o + p
        to_r = _bf(to.reshape(CH, 128, 17).transpose(1, 0, 2).reshape(128, CH * 17))
        in_maps.append({"xc": xc, "ta": ta, "to": to_r})
    return in_maps


def run_cores(in_maps, trace=False, repeat=1, **kw):
    nc = build_program(repeat)
    return run_bass_kernel_spmd(nc, in_maps, list(range(NCORES)),
                                trace=trace, **kw)


def kernel(x, train_inputs, train_outputs, spread):
    in_maps = host_prep(x, train_inputs, train_outputs, spread)
    res = run_cores(in_maps)
    total = np.zeros((17, B), dtype=np.float64)
    for c in range(NCORES):
        total += res.results[c]["out"].astype(np.float64)
    out = (total[:16] / total[16]).T.astype(np.float32)
    return out
